# revision 1
# baseline (speedup 1.0000x reference)
"""Trainium2 Bass kernel for single-head attention (B=8, S=2048, D=U=512).

Sharding: data-parallel over batch — one batch element per NeuronCore (8 cores).

Per-core dataflow (all matmuls in float32r):
  1. PE-transpose query/value into XT/VT ([d on partitions, s free]).
  2. Projections: qT = W1^T X^T, kT = W2^T V^T  ([u part, s free]);
     v = V W3 natural ([s part, u free]).
  3. Attention over i-blocks of 512 query positions:
     scoresT[j, i] = sum_u kT[u,j] qT[u,i]   (PSUM, accumulated over u-chunks)
     expS = exp(scoresT / sqrt(U))           (ScalarE, written f32r to SBUF)
     ctx[i, u]  = sum_j expS[j,i] v[j,u]     (PE, expS chunks stationary)
     den[i]     = sum_j expS[j,i]            (PE, all-ones stationary, expS
                                              moving; row transposed to a
                                              per-partition column via PE)
     out[i, :]  = ctx[i, :] / den[i]         (DVE reciprocal + tensor_scalar)
  Softmax max-subtraction is skipped: scores ~ N(0,1), exp() cannot overflow.
"""

import math
import os
import sys

for _p in ("/opt/trn_rl_repo", os.path.expanduser("~/.axon_site/_ro/trn_rl_repo")):
    if os.path.isdir(_p) and _p not in sys.path:
        sys.path.insert(0, _p)

import numpy as np

import concourse.bass as bass
import concourse.tile as tile
from concourse import bacc, mybir
from concourse.bass import ts
from concourse.bass_utils import run_bass_kernel_spmd
from concourse.masks import make_identity

F32 = mybir.dt.float32
F32R = mybir.dt.float32r
EXP = mybir.ActivationFunctionType.Exp

P = 128          # partitions
B = 8            # batch (one element per core)
S = 2048         # sequence length
D = 512          # model dim
U = 512          # units
DC = D // P      # 4 d-chunks
UC = U // P      # 4 u-chunks
SC = S // P      # 16 s-chunks
IB = 512         # i-block (query positions per attention block)
NIB = S // IB    # 4
ICC = IB // P    # 4 i-chunks per block
SCALE = 1.0 / math.sqrt(float(U))


def _emit(nc, tc, q_d, v_d, w1_d, w2_d, w3_d, o_d):
    with tc.tile_pool(name="const", bufs=1) as cp:
        ident = cp.tile([P, P], F32, name="ident")
        make_identity(nc, ident)
        # Stationary all-ones operand for softmax denominators: [K=128, M=2]
        # (fp32r needs even sizes; only output row 0 is used).
        ones32 = cp.tile([P, 2], F32, name="ones32")
        nc.vector.memset(ones32, 1.0)
        ones = cp.tile([P, 2], F32R, name="ones")
        nc.vector.tensor_copy(ones, ones32)

        with tc.tile_pool(name="wpool", bufs=1) as wp:
            w1 = wp.tile([P, DC, U], F32R, name="w1")
            w2 = wp.tile([P, DC, U], F32R, name="w2")
            w3 = wp.tile([P, DC, U], F32R, name="w3")

            with tc.tile_pool(name="qkv", bufs=1) as qkvp:
                qT = qkvp.tile([P, UC, S], F32R, name="qT")
                kT = qkvp.tile([P, UC, S], F32R, name="kT")
                vN = qkvp.tile([P, SC, U], F32R, name="vN")

                # ---- phase 1: transposes + projections ----
                with tc.tile_pool(name="xtp", bufs=1) as xtp, \
                     tc.tile_pool(name="loadp", bufs=2) as loadp, \
                     tc.tile_pool(name="tps", bufs=4, space="PSUM") as tpsp, \
                     tc.tile_pool(name="pjps", bufs=4, space="PSUM") as pjps:
                    xT = xtp.tile([P, DC, S], F32R, name="xT")
                    vT = xtp.tile([P, DC, S], F32R, name="vT")

                    # PSUM->SBUF copies alternate between DVE and ACT so
                    # neither engine gates the PE transpose/matmul stream.
                    _cp_idx = [0]

                    def copy_out(dst, src):
                        _cp_idx[0] += 1
                        if _cp_idx[0] % 2:
                            nc.vector.tensor_copy(dst, src)
                        else:
                            nc.scalar.copy(dst, src)

                    def transpose_chunk(nat, dstT, sc):
                        # 4 transposes share one PSUM bank; single copy out
                        tp = tpsp.tile([P, DC * P], F32, tag="tp")
                        for dc in range(DC):
                            nc.tensor.transpose(
                                tp[:, ts(dc, P)], nat[:, ts(dc, P)].bitcast(F32),
                                ident)
                        copy_out(dstT[:, :, ts(sc, P)],
                                 tp.rearrange("p (c q) -> p c q", c=DC))

                    def emit_vn(jc):
                        ps = pjps.tile([P, U], F32, tag="pj")
                        for dc in range(DC):
                            nc.tensor.matmul(
                                ps, vT[:, dc, ts(jc, P)], w3[:, dc, :],
                                start=(dc == 0), stop=(dc == DC - 1))
                        copy_out(vN[:, jc, :], ps)

                    def emit_kt(ib):
                        for uc in range(UC):
                            ps = pjps.tile([P, IB], F32, tag="pj")
                            for dc in range(DC):
                                nc.tensor.matmul(
                                    ps, w2[:, dc, ts(uc, P)],
                                    vT[:, dc, ts(ib, IB)],
                                    start=(dc == 0), stop=(dc == DC - 1))
                            copy_out(kT[:, uc, ts(ib, IB)], ps)

                    def emit_qt(ib):
                        for uc in range(UC):
                            ps = pjps.tile([P, IB], F32, tag="pj")
                            for dc in range(DC):
                                nc.tensor.matmul(
                                    ps, w1[:, dc, ts(uc, P)],
                                    xT[:, dc, ts(ib, IB)],
                                    start=(dc == 0), stop=(dc == DC - 1))
                            copy_out(qT[:, uc, ts(ib, IB)], ps)

                    # Interleave DMA arrival with PE work. Projections run one
                    # chunk behind the transposes so the PSUM->SBUF copy of
                    # chunk jc completes while PE transposes chunk jc+1.
                    nc.sync.dma_start(w3, w3_d.rearrange("(c p) u -> p c u", p=P))
                    for jc in range(SC):
                        if jc % 4 == 0:
                            nat4 = loadp.tile([P, 4, D], F32R, tag="nat",
                                              name=f"nat_v{jc // 4}")
                            nc.sync.dma_start(
                                nat4, v_d[ts(jc // 4, 4 * P), :].rearrange(
                                    "(c p) d -> p c d", p=P))
                        if jc == 1:
                            nc.sync.dma_start(
                                w2, w2_d.rearrange("(c p) u -> p c u", p=P))
                        transpose_chunk(nat4[:, jc % 4, :], vT, jc)
                        if jc > 0:
                            emit_vn(jc - 1)
                        if jc % 4 == 0 and jc > 0:
                            emit_kt(jc // 4 - 1)
                    emit_vn(SC - 1)
                    # X side: transpose each chunk; qT one i-block behind
                    for sc in range(SC):
                        if sc % 4 == 0:
                            nat4 = loadp.tile([P, 4, D], F32R, tag="nat",
                                              name=f"nat_x{sc // 4}")
                            nc.sync.dma_start(
                                nat4, q_d[ts(sc // 4, 4 * P), :].rearrange(
                                    "(c p) d -> p c d", p=P))
                        if sc == 1:
                            nc.sync.dma_start(
                                w1, w1_d.rearrange("(c p) u -> p c u", p=P))
                        transpose_chunk(nat4[:, sc % 4, :], xT, sc)
                        if sc == 0:
                            emit_kt(NIB - 1)
                        if sc % 4 == 0 and sc > 0:
                            emit_qt(sc // 4 - 1)
                    emit_qt(NIB - 1)

                # ---- phase 2: attention ----
                with tc.tile_pool(name="expp", bufs=2) as expp, \
                     tc.tile_pool(name="scps", bufs=2, space="PSUM") as scps, \
                     tc.tile_pool(name="ctps", bufs=2, space="PSUM") as ctps, \
                     tc.tile_pool(name="dnps", bufs=2, space="PSUM") as dnps, \
                     tc.tile_pool(name="tdps", bufs=2, space="PSUM") as tdps, \
                     tc.tile_pool(name="outp", bufs=3) as outp:
                    for ib in range(NIB):
                        expB = expp.tile([P, SC, IB], F32R, name="expB")
                        # denT[0, i] accumulates sum_j expS[j, i] for this
                        # i-block (ones is the 2-col stationary; row 1 unused)
                        denT = dnps.tile([2, IB], F32, tag="dn")
                        for jc in range(SC):
                            ps = scps.tile([P, IB], F32, tag="sc")
                            for uc in range(UC):
                                nc.tensor.matmul(
                                    ps, kT[:, uc, ts(jc, P)], qT[:, uc, ts(ib, IB)],
                                    start=(uc == 0), stop=(uc == UC - 1))
                            nc.scalar.activation(expB[:, jc, :], ps, EXP, scale=SCALE)
                        for jc in range(SC):
                            nc.tensor.matmul(
                                denT, ones, expB[:, jc, :],
                                start=(jc == 0), stop=(jc == SC - 1))
                        # denominator row -> per-partition column via PE
                        # transpose of 128-wide slices
                        denTs = outp.tile([1, IB], F32, tag="denTs")
                        nc.vector.tensor_copy(denTs, denT[0:1, :])
                        dcol = tdps.tile([P, ICC], F32, tag="dcol")
                        for icc in range(ICC):
                            nc.tensor.transpose(
                                dcol[:, icc:icc + 1], denTs[0:1, ts(icc, P)],
                                ident[0:1, 0:1])
                        for icc in range(ICC):
                            i_glob = ib * ICC + icc
                            recip = outp.tile([P, 1], F32, tag="recip")
                            nc.vector.reciprocal(recip, dcol[:, icc:icc + 1])
                            cps = ctps.tile([P, U], F32, tag="ct")
                            for jc in range(SC):
                                nc.tensor.matmul(
                                    cps, expB[:, jc, ts(icc, P)], vN[:, jc, :],
                                    start=(jc == 0), stop=(jc == SC - 1))
                            co = outp.tile([P, U], F32, tag="co")
                            nc.vector.tensor_scalar_mul(co, cps, recip)
                            nc.sync.dma_start(o_d[ts(i_glob, P), :], co)


_PROGRAM = None


def _get_program():
    global _PROGRAM
    if _PROGRAM is None:
        nc = bacc.Bacc("TRN2", target_bir_lowering=False, debug=False,
                       num_devices=B)
        q_d = nc.dram_tensor("query", (S, D), F32R, kind="ExternalInput").ap()
        v_d = nc.dram_tensor("value", (S, D), F32R, kind="ExternalInput").ap()
        w1_d = nc.dram_tensor("W1", (D, U), F32R, kind="ExternalInput").ap()
        w2_d = nc.dram_tensor("W2", (D, U), F32R, kind="ExternalInput").ap()
        w3_d = nc.dram_tensor("W3", (D, U), F32R, kind="ExternalInput").ap()
        o_d = nc.dram_tensor("out", (S, U), F32, kind="ExternalOutput").ap()
        with tile.TileContext(nc) as tc:
            _emit(nc, tc, q_d, v_d, w1_d, w2_d, w3_d, o_d)
        nc.compile()
        _PROGRAM = nc
    return _PROGRAM


def kernel(**inputs) -> np.ndarray:
    query = np.ascontiguousarray(inputs["query"], dtype=np.float32)
    value = np.ascontiguousarray(inputs["value"], dtype=np.float32)
    W1 = np.ascontiguousarray(inputs["W1"], dtype=np.float32)
    W2 = np.ascontiguousarray(inputs["W2"], dtype=np.float32)
    W3 = np.ascontiguousarray(inputs["W3"], dtype=np.float32)
    assert query.shape == (B, S, D) and value.shape == (B, S, D)

    nc = _get_program()
    in_maps = [
        {"query": query[b], "value": value[b], "W1": W1, "W2": W2, "W3": W3}
        for b in range(B)
    ]
    res = run_bass_kernel_spmd(nc, in_maps, core_ids=list(range(B)))
    return np.stack([res.results[b]["out"] for b in range(B)], axis=0)



# revision 8
# speedup vs baseline: 1.1785x; 1.1785x over previous
"""Trainium2 Bass kernel for single-head attention (B=8, S=2048, D=U=512).

Sharding: data-parallel over batch - one batch element per NeuronCore (8 cores).

Math: score = X W1 (V W2)^T / sqrt(U) = X M V^T with M = W1 W2^T folded once
per core (saves one full projection). context = softmax(score) (V W3).

Per-core dataflow:
  Phase 0/1 (streamed with DMA):
    - W1,W2 PE-transposed (f32r); M = W1^T^T... M[d,e] = sum_u W1[d,u]W2[e,u]
      computed on PE, scaled by 16, stored as fp8e4 hi/lo pair (m12_8).
    - W3 scaled by 16 stored as fp8e4 hi/lo (w3_8).
    - V chunks PE-transposed to vT (f32r psum) then quantized to fp8 hi/lo
      (vT8). vN[j,u] = 16*(V W3)[j,u] via fp8 DoubleRow matmuls
      (3-term hi/lo cross products), stored f32r with an extra column 512
      holding 16.0 (fused softmax denominator) and zero padding.
    - X chunks PE-transposed, quantized to xT8 hi/lo; qmT = 16*(M^T x^T)
      via DoubleRow, quantized to qmT8 hi/lo.
  Phase 2 (attention, pipelined over i-blocks of 512 query positions):
    scoresT[j,i] = sum_e vT[e,j] qmT[e,i] via fp8 DoubleRow (3-term hi/lo),
      psum = 16 * unscaled score.
    expB = exp(psum * SCALE/16) on ACT (f32r).
    ctx[i,u] (+den in col 512) = sum_j expB[j,i] vN[j,u] f32r matmuls in two
      psum groups (258+256 cols <= 512-col psum bank).
    out = ctx * (1/den) via DVE reciprocal + DVE/ACT scalar-mul, DMA to HBM.

fp8 DoubleRow (both operands fp8e4, paired along a leading free dim of 2)
processes 2 contraction rows x 2 columns per cycle: 0.5 cycles/output-col vs
1.0 for f32r, at 256-deep contraction per call. The hi/lo 3-call scheme
(hi*hi, hi*lo, lo*hi) restores ~8-bit per-term accuracy, measured end-to-end
rel err ~<1e-2 against the f32 reference (threshold 2e-2).
"""

import math
import os
import sys

for _p in ("/opt/trn_rl_repo", os.path.expanduser("~/.axon_site/_ro/trn_rl_repo")):
    if os.path.isdir(_p) and _p not in sys.path:
        sys.path.insert(0, _p)

import numpy as np

import concourse.bass as bass
import concourse.tile as tile
from concourse import bacc, mybir
from concourse.bass import ts
from concourse.bass_utils import run_bass_kernel_spmd
from concourse.masks import make_identity

F32 = mybir.dt.float32
F32R = mybir.dt.float32r
F8 = mybir.dt.float8e4
EXP = mybir.ActivationFunctionType.Exp
DR = mybir.MatmulPerfMode.DoubleRow
MUL = mybir.AluOpType.mult
SUB = mybir.AluOpType.subtract

P = 128          # partitions
B = 8            # batch (one element per core)
S = 2048         # sequence length
D = 512          # model dim
U = 512          # units
DC = D // P      # 4 chunks of the contraction dims
SC = S // P      # 16 s-chunks
IB = 512         # i-block (query positions per attention block)
NIB = S // IB    # 4
ICC = IB // P    # 4 i-chunks per block
SCALE = 1.0 / math.sqrt(float(U))
WSC = 16.0       # weight pre-scale so fp8 quantization stays in normal range
VNF = 520        # vN free width: 512 u-cols + col 512 = WSC (den) + pad
CA = 258         # ctx psum group A columns (u 0..257)
CB = 256         # ctx psum group B columns (u 258..511, den, pad)

# DoubleRow hi/lo call list: (stationary half, moving half)
HL3 = ((0, 0), (0, 1), (1, 0))


def _emit(nc, tc, q_d, v_d, w1_d, w2_d, w3_d, o_d):
    with tc.tile_pool(name="const", bufs=1) as cp:
        identf = cp.tile([P, P], F32, name="identf")
        make_identity(nc, identf)
        ident = cp.tile([P, P], F32R, name="ident")
        nc.vector.tensor_copy(ident, identf)

        with tc.tile_pool(name="persist", bufs=1) as pp:
            m12_8 = pp.tile([P, 2, DC, D], F8, name="m12_8")
            w3_8 = pp.tile([P, 2, DC, U], F8, name="w3_8")

            with tc.tile_pool(name="qkv", bufs=1) as qkvp:
                vT8 = qkvp.tile([P, 2, DC, S], F8, name="vT8")
                xT8 = qkvp.tile([P, 2, DC, S], F8, name="xT8")
                qmT8 = qkvp.tile([P, 2, DC, S], F8, name="qmT8")
                vN = qkvp.tile([P, SC, VNF], F32R, name="vN")
                # den column = WSC (vN holds 16*v so num/den scales cancel);
                # memset on f32r fails ISA checks, so stage in f32 and copy.
                dtmp = cp.tile([P, SC, VNF - U], F32, name="dtmp")
                nc.gpsimd.memset(dtmp, 0.0)
                nc.gpsimd.memset(dtmp[:, :, 0:1], WSC)
                nc.vector.tensor_copy(vN[:, :, U:VNF], dtmp)

                _phase1(nc, tc, q_d, v_d, w1_d, w2_d, w3_d,
                        ident, m12_8, w3_8, vT8, xT8, qmT8, vN)
                _phase2(nc, tc, o_d, vT8, qmT8, vN)


def _phase1(nc, tc, q_d, v_d, w1_d, w2_d, w3_d,
            ident, m12_8, w3_8, vT8, xT8, qmT8, vN):
    with tc.tile_pool(name="wtmp", bufs=1) as wp, \
         tc.tile_pool(name="loadp", bufs=2) as loadp, \
         tc.tile_pool(name="tps", bufs=2, space="PSUM") as tpsp, \
         tc.tile_pool(name="pjps", bufs=3, space="PSUM") as pjps:
        w1n = wp.tile([P, DC, U], F32R, name="w1n")
        w2n = wp.tile([P, DC, U], F32R, name="w2n")
        w3n = wp.tile([P, DC, U], F32R, name="w3n")
        w1t = wp.tile([P, DC, D], F32R, name="w1t")
        w2t = wp.tile([P, DC, D], F32R, name="w2t")

        _cp = [0]

        def copy_f32r(dst, src):
            # ACT: DVE is saturated by the fp8 residual (stt) ops
            nc.scalar.copy(dst, src)

        def hilo(dst8, hl_idx, src, scale=1.0, lo_eng=None):
            # dst8[...,0,...] = fp8(scale*src); dst8[...,1,...] = residual.
            hi = dst8[tuple([slice(None), 0] + hl_idx)]
            lo = dst8[tuple([slice(None), 1] + hl_idx)]
            if scale == 1.0:
                nc.scalar.copy(hi, src)
            else:
                nc.scalar.mul(hi, src, scale)
            (lo_eng or nc.vector).scalar_tensor_tensor(
                lo, src, scale, hi, op0=MUL, op1=SUB)

        def transpose_chunk(nat, sc):
            # 4 transposes of 128x128 blocks into one f32r psum tile
            tp = tpsp.tile([P, DC, P], F32R, tag="tp")
            for dc in range(DC):
                nc.tensor.transpose(tp[:, dc, :], nat[:, ts(dc, P)], ident)
            return tp

        def emit_wt(wn, wt):
            # wt[u%128, uc, d] = W[d, u] transposed blocks
            for uc in range(DC):
                tp = tpsp.tile([P, DC, P], F32R, tag="tp")
                for dc in range(DC):
                    nc.tensor.transpose(tp[:, dc, :], wn[:, dc, ts(uc, P)], ident)
                copy_f32r(wt[:, uc, :].rearrange("p (c q) -> p c q", c=DC), tp)

        def emit_m12(dc):
            # M[d,e] = sum_u W1[d,u] W2[e,u], rows dc*128..dc*128+127
            ps = pjps.tile([P, D], F32, tag="pj")
            for uc in range(DC):
                nc.tensor.matmul(ps, w1t[:, uc, ts(dc, P)], w2t[:, uc, :],
                                 start=(uc == 0), stop=(uc == DC - 1))
            hilo(m12_8, [dc, slice(None)], ps, scale=WSC)

        def dr6(ps, stat8, s_idx, mov8, m_idx):
            # 6 DoubleRow calls: 2 pair-chunks x (hh, hl, lh)
            n = 0
            for pc in (0, 2):
                for sh, mh in HL3:
                    n += 1
                    nc.tensor.matmul(
                        ps,
                        stat8[tuple([slice(None), sh, slice(pc, pc + 2)] + s_idx)],
                        mov8[tuple([slice(None), mh, slice(pc, pc + 2)] + m_idx)],
                        start=(n == 1), stop=(n == 6), perf_mode=DR)

        def emit_vn(jc):
            # vN[j,u] = 16 * sum_d V[j,d] W3[d,u]
            ps = pjps.tile([P, U], F32, tag="pj")
            dr6(ps, vT8, [ts(jc, P)], w3_8, [slice(None)])
            copy_f32r(vN[:, jc, 0:U], ps)

        def emit_qmt(ib):
            # qmT[e,i] = 16 * sum_d M[d,e] xT[d,i] for i-block ib
            for ec in range(DC):
                ps = pjps.tile([P, IB], F32, tag="pj")
                dr6(ps, m12_8, [ts(ec, P)], xT8, [ts(ib, IB)])
                hilo(qmT8, [ec, ts(ib, IB)], ps)

        # ---- streaming schedule ----
        # Serial DMA queue: V0, W3, V1, W1, V2, W2, V3, X0..X3.
        # W3 early (all vN matmuls need it); W2 late (only M12 -> qmT need it).
        nat_v = []
        nat_v.append(loadp.tile([P, 4, D], F32R, tag="nat", name="nat_v0"))
        nc.sync.dma_start(nat_v[0],
                          v_d[ts(0, 4 * P), :].rearrange("(c p) d -> p c d", p=P))
        nc.sync.dma_start(w3n, w3_d.rearrange("(c p) u -> p c u", p=P))

        vn_done = 0
        for jc in range(SC):
            g = jc // 4
            if jc % 4 == 0 and g + 1 < 4:
                t = loadp.tile([P, 4, D], F32R, tag="nat", name=f"nat_v{g + 1}")
                nc.sync.dma_start(
                    t, v_d[ts(g + 1, 4 * P), :].rearrange("(c p) d -> p c d", p=P))
                nat_v.append(t)
                if g == 0:
                    nc.sync.dma_start(w1n, w1_d.rearrange("(c p) u -> p c u", p=P))
                elif g == 1:
                    nc.sync.dma_start(w2n, w2_d.rearrange("(c p) u -> p c u", p=P))
            tp = transpose_chunk(nat_v[g][:, jc % 4, :], jc)
            hilo(vT8, [slice(None), ts(jc, P)], tp)
            if jc == 3:
                hilo(w3_8, [slice(None), slice(None)], w3n, scale=WSC)
            elif jc == 7:
                emit_wt(w1n, w1t)
            elif jc == 11:
                emit_wt(w2n, w2t)
            elif jc >= 12:
                emit_m12(jc - 12)
            if jc >= 4:
                emit_vn(vn_done)
                vn_done += 1

        # ---- X stream ----
        for sc in range(SC):
            g = sc // 4
            if sc % 4 == 0:
                nat_x = loadp.tile([P, 4, D], F32R, tag="nat", name=f"nat_x{g}")
                nc.sync.dma_start(
                    nat_x, q_d[ts(g, 4 * P), :].rearrange("(c p) d -> p c d", p=P))
            tp = transpose_chunk(nat_x[:, sc % 4, :], sc)
            hilo(xT8, [slice(None), ts(sc, P)], tp)
            if vn_done < SC:
                emit_vn(vn_done)
                vn_done += 1
            if sc % 4 == 3:
                emit_qmt(g)


def _phase2(nc, tc, o_d, vT8, qmT8, vN):
    with tc.tile_pool(name="expp", bufs=2) as expp, \
         tc.tile_pool(name="outp", bufs=4) as outp, \
         tc.tile_pool(name="scps", bufs=2, space="PSUM") as scps, \
         tc.tile_pool(name="caps", bufs=2, space="PSUM") as caps, \
         tc.tile_pool(name="cbps", bufs=2, space="PSUM") as cbps:

        def emit_scores(ib):
            expB = expp.tile([P, SC, IB], F32R, name="expB")
            for jc in range(SC):
                ps = scps.tile([P, IB], F32, tag="sc")
                n = 0
                for pc in (0, 2):
                    for sh, mh in HL3:
                        n += 1
                        nc.tensor.matmul(
                            ps,
                            vT8[:, sh, pc:pc + 2, ts(jc, P)],
                            qmT8[:, mh, pc:pc + 2, ts(ib, IB)],
                            start=(n == 1), stop=(n == 6), perf_mode=DR)
                nc.scalar.activation(expB[:, jc, :], ps, EXP, scale=SCALE / WSC)
            return expB

        def emit_ctx(expB, ib):
            for icc in range(ICC):
                i_glob = ib * ICC + icc
                psA = caps.tile([P, CA], F32, tag="ca")
                psB = cbps.tile([P, CB], F32, tag="cb")
                for jc in range(SC):
                    st = expB[:, jc, ts(icc, P)]
                    nc.tensor.matmul(psA, st, vN[:, jc, 0:CA],
                                     start=(jc == 0), stop=(jc == SC - 1))
                    nc.tensor.matmul(psB, st, vN[:, jc, CA:CA + CB],
                                     start=(jc == 0), stop=(jc == SC - 1))
                # den (= 16*sum_j exp) sits at psB col U-CA (=254)
                rec = outp.tile([P, 1], F32, tag="rec")
                nc.vector.reciprocal(rec, psB[:, U - CA:U - CA + 1])
                co = outp.tile([P, U], F32, tag="co")
                nc.vector.tensor_scalar_mul(co[:, 0:CA], psA, rec)
                nc.scalar.mul(co[:, CA:U], psB[:, 0:U - CA], rec)
                nc.sync.dma_start(o_d[ts(i_glob, P), :], co)

        prev = None
        for ib in range(NIB):
            expB = emit_scores(ib)
            if prev is not None:
                emit_ctx(*prev)
            prev = (expB, ib)
        emit_ctx(*prev)


_PROGRAM = None


def _get_program():
    global _PROGRAM
    if _PROGRAM is None:
        nc = bacc.Bacc("TRN2", target_bir_lowering=False, debug=False,
                       num_devices=B)
        q_d = nc.dram_tensor("query", (S, D), F32R, kind="ExternalInput").ap()
        v_d = nc.dram_tensor("value", (S, D), F32R, kind="ExternalInput").ap()
        w1_d = nc.dram_tensor("W1", (D, U), F32R, kind="ExternalInput").ap()
        w2_d = nc.dram_tensor("W2", (D, U), F32R, kind="ExternalInput").ap()
        w3_d = nc.dram_tensor("W3", (D, U), F32R, kind="ExternalInput").ap()
        o_d = nc.dram_tensor("out", (S, U), F32, kind="ExternalOutput").ap()
        with tile.TileContext(nc) as tc:
            _emit(nc, tc, q_d, v_d, w1_d, w2_d, w3_d, o_d)
        nc.compile()
        _PROGRAM = nc
    return _PROGRAM


def kernel(**inputs) -> np.ndarray:
    query = np.ascontiguousarray(inputs["query"], dtype=np.float32)
    value = np.ascontiguousarray(inputs["value"], dtype=np.float32)
    W1 = np.ascontiguousarray(inputs["W1"], dtype=np.float32)
    W2 = np.ascontiguousarray(inputs["W2"], dtype=np.float32)
    W3 = np.ascontiguousarray(inputs["W3"], dtype=np.float32)
    assert query.shape == (B, S, D) and value.shape == (B, S, D)

    nc = _get_program()
    in_maps = [
        {"query": query[b], "value": value[b], "W1": W1, "W2": W2, "W3": W3}
        for b in range(B)
    ]
    res = run_bass_kernel_spmd(nc, in_maps, core_ids=list(range(B)))
    return np.stack([res.results[b]["out"] for b in range(B)], axis=0)


# revision 10
# speedup vs baseline: 1.2263x; 1.0406x over previous
"""Trainium2 Bass kernel for single-head attention (B=8, S=2048, D=U=512).

Sharding: data-parallel over batch - one batch element per NeuronCore (8 cores).

Math: score = X W1 (V W2)^T / sqrt(U) = X M V^T with M = W1 W2^T folded once
per core (saves one full projection). context = softmax(score) (V W3).

Dataflow per core:
  Phase 1 (streamed with the serial DMA queue: V0a,V0b,W3,V1,W1,V2,W2,V3,X0):
    - V chunks PE-transposed (f32r) and quantized to fp8e4 hi/lo (vT8).
    - W1,W2 PE-transposed; M = W1 W2^T on PE, scaled by 16, fp8 hi/lo (m12_8).
      W3 scaled by 16, fp8 hi/lo (w3_8).
    - vN[j,u] = 16*(V W3) via fp8 DoubleRow (3-term hi/lo cross products),
      f32r, with column 512 = 16.0 (fused softmax denominator).
    - X group 0 transposed/quantized (xT8); qmT(0) = 16*(M^T x^T) via
      DoubleRow, quantized hi/lo (qmT8).
  Phase 2 per i-block ib (pipelined):
    scores(ib): scoresT[j,i] = sum_e vT[e,j] qmT[e,i] via fp8 DoubleRow;
      exp on ACT -> expB (f32r). X group ib+1 transpose + qmT(ib+1) are
      interleaved here (PE) so their vector ops land in phase-2 ACT/DVE slack.
    ctx(ib-1): ctx[i,u] (+den col) = sum_j expB[j,i] vN[j,u], f32r matmuls in
      two psum groups (258+256 cols); out = ctx * recip(den) -> DMA.

fp8 DoubleRow (operands paired along a leading free dim of 2) runs at 0.5
cycles/output-col with 256-deep contraction per call - 4x f32r throughput;
the 3-call hi/lo scheme nets 2.67x at ~8-bit per-term accuracy. Measured
end-to-end rel err ~3e-3 (threshold 2e-2).
"""

import math
import os
import sys

for _p in ("/opt/trn_rl_repo", os.path.expanduser("~/.axon_site/_ro/trn_rl_repo")):
    if os.path.isdir(_p) and _p not in sys.path:
        sys.path.insert(0, _p)

import numpy as np

import concourse.bass as bass
import concourse.tile as tile
from concourse import bacc, mybir
from concourse.bass import ts
from concourse.bass_utils import run_bass_kernel_spmd
from concourse.masks import make_identity

F32 = mybir.dt.float32
F32R = mybir.dt.float32r
F8 = mybir.dt.float8e4
EXP = mybir.ActivationFunctionType.Exp
DR = mybir.MatmulPerfMode.DoubleRow
MUL = mybir.AluOpType.mult
SUB = mybir.AluOpType.subtract

P = 128          # partitions
B = 8            # batch (one element per core)
S = 2048         # sequence length
D = 512          # model dim
U = 512          # units
DC = D // P      # 4 chunks of the contraction dims
SC = S // P      # 16 s-chunks
IB = 512         # i-block (query positions per attention block)
NIB = S // IB    # 4
ICC = IB // P    # 4 i-chunks per block
SCALE = 1.0 / math.sqrt(float(U))
WSC = 16.0       # weight pre-scale so fp8 quantization stays in normal range
VNF = 520        # vN free width: 512 u-cols + col 512 = WSC (den) + pad
CA = 258         # ctx psum group A columns (u 0..257)
CB = 256         # ctx psum group B columns (u 258..511, den at 254, pad)

# DoubleRow hi/lo call list: (stationary half, moving half)
HL3 = ((0, 0), (0, 1), (1, 0))


def _emit(nc, tc, q_d, v_d, w1_d, w2_d, w3_d, o_d):
    with tc.tile_pool(name="const", bufs=1) as cp, \
         tc.tile_pool(name="persist", bufs=1) as pp:
        identf = cp.tile([P, P], F32, name="identf")
        make_identity(nc, identf)
        ident = cp.tile([P, P], F32R, name="ident")
        nc.vector.tensor_copy(ident, identf)

        m12_8 = pp.tile([P, 2, DC, D], F8, name="m12_8")
        w3_8 = pp.tile([P, 2, DC, U], F8, name="w3_8")
        vT8 = pp.tile([P, 2, DC, S], F8, name="vT8")
        xT8 = pp.tile([P, 2, DC, S], F8, name="xT8")
        qmT8 = pp.tile([P, 2, DC, S], F8, name="qmT8")
        vN = pp.tile([P, SC, VNF], F32R, name="vN")
        # den column = WSC (vN holds 16*v so num/den scales cancel); memset
        # on f32r fails ISA checks, so stage in f32 and copy.
        dtmp = cp.tile([P, SC, VNF - U], F32, name="dtmp")
        nc.gpsimd.memset(dtmp, 0.0)
        nc.gpsimd.memset(dtmp[:, :, 0:1], WSC)
        nc.vector.tensor_copy(vN[:, :, U:VNF], dtmp)

        _veng = [0]

        def hilo(dst8, hl_idx, src, scale=1.0):
            # dst8[...,0,...] = fp8(scale*src) on ACT;
            # dst8[...,1,...] = fp8(scale*src - hi) on DVE.
            hi = dst8[tuple([slice(None), 0] + hl_idx)]
            lo = dst8[tuple([slice(None), 1] + hl_idx)]
            if scale == 1.0:
                nc.scalar.copy(hi, src)
            else:
                nc.scalar.mul(hi, src, scale)
            nc.vector.scalar_tensor_tensor(lo, src, scale, hi, op0=MUL, op1=SUB)

        def copy_f32r(dst, src):
            _veng[0] += 1
            if _veng[0] % 2:
                nc.scalar.copy(dst, src)
            else:
                nc.vector.tensor_copy(dst, src)

        def dr6(ps, stat8, s_idx, mov8, m_idx):
            # 6 DoubleRow calls: 2 pair-chunks x (hh, hl, lh)
            n = 0
            for pc in (0, 2):
                for sh, mh in HL3:
                    n += 1
                    nc.tensor.matmul(
                        ps,
                        stat8[tuple([slice(None), sh, slice(pc, pc + 2)] + s_idx)],
                        mov8[tuple([slice(None), mh, slice(pc, pc + 2)] + m_idx)],
                        start=(n == 1), stop=(n == 6), perf_mode=DR)

        _phase1(nc, tc, q_d, v_d, w1_d, w2_d, w3_d, ident,
                m12_8, w3_8, vT8, xT8, qmT8, vN, hilo, copy_f32r, dr6)
        _phase2(nc, tc, q_d, o_d, ident, m12_8, vT8, xT8, qmT8, vN, hilo, dr6)


def _phase1(nc, tc, q_d, v_d, w1_d, w2_d, w3_d, ident,
            m12_8, w3_8, vT8, xT8, qmT8, vN, hilo, copy_f32r, dr6):
    with tc.tile_pool(name="wtmp", bufs=1) as wp, \
         tc.tile_pool(name="loadp", bufs=3) as loadp, \
         tc.tile_pool(name="tps", bufs=2, space="PSUM") as tpsp, \
         tc.tile_pool(name="pjps", bufs=2, space="PSUM") as pjps:
        w1n = wp.tile([P, DC, U], F32R, name="w1n")
        w2n = wp.tile([P, DC, U], F32R, name="w2n")
        w3n = wp.tile([P, DC, U], F32R, name="w3n")
        w1t = wp.tile([P, DC, D], F32R, name="w1t")
        w2t = wp.tile([P, DC, D], F32R, name="w2t")

        def transpose_pair(nat0, nat1, dst8, jc):
            # two 128-row chunks -> one 2-bank psum tile (layout [c, g, q] so
            # (g q) merges to a contiguous 256-wide dim) -> one hi + one lo op
            tp = tpsp.tile([P, DC, 2, P], F32R, tag="tp")
            for g, nat in ((0, nat0), (1, nat1)):
                for dc in range(DC):
                    nc.tensor.transpose(tp[:, dc, g, :], nat[:, ts(dc, P)], ident)
            src = tp.rearrange("p c g q -> p c (g q)")
            hi = dst8[:, 0, :, jc * P:(jc + 2) * P]
            lo = dst8[:, 1, :, jc * P:(jc + 2) * P]
            nc.scalar.copy(hi, src)
            nc.vector.scalar_tensor_tensor(lo, src, 1.0, hi, op0=MUL, op1=SUB)

        def emit_wt(wn, wt):
            # wt[u%128, uc, d] = W[d, u] transposed blocks
            for ucp in (0, 2):
                tp = tpsp.tile([P, DC, 2, P], F32R, tag="tp")
                for g in (0, 1):
                    for dc in range(DC):
                        nc.tensor.transpose(tp[:, dc, g, :],
                                            wn[:, dc, ts(ucp + g, P)], ident)
                    copy_f32r(
                        wt[:, ucp + g, :].rearrange("p (k q) -> p k q", k=DC),
                        tp[:, :, g, :])

        def emit_m12(dcp):
            # M[d,e] = sum_u W1[d,u] W2[e,u], row-chunks (2dcp, 2dcp+1)
            ps = pjps.tile([P, 2, D], F32, tag="pj")
            for g in (0, 1):
                for uc in range(DC):
                    nc.tensor.matmul(ps[:, g, :], w1t[:, uc, ts(2 * dcp + g, P)],
                                     w2t[:, uc, :],
                                     start=(uc == 0), stop=(uc == DC - 1))
                hilo(m12_8, [2 * dcp + g, slice(None)], ps[:, g, :], scale=WSC)

        def emit_vn2(jc):
            # vN[j,u] = 16 * sum_d V[j,d] W3[d,u], two j-chunks at once
            ps = pjps.tile([P, 2, U], F32, tag="pj")
            dr6(ps[:, 0, :], vT8, [ts(jc, P)], w3_8, [slice(None)])
            dr6(ps[:, 1, :], vT8, [ts(jc + 1, P)], w3_8, [slice(None)])
            copy_f32r(vN[:, jc:jc + 2, 0:U], ps)

        def emit_xt(sc, nat):
            tp = tpsp.tile([P, DC, 2, P], F32R, tag="tp")
            for dc in range(DC):
                nc.tensor.transpose(tp[:, dc, 0, :], nat[:, ts(dc, P)], ident)
            hilo(xT8, [slice(None), ts(sc, P)], tp[:, :, 0, :])

        def emit_qmt(ib):
            # qmT[e,i] = 16 * sum_d M[d,e] xT[d,i] for i-block ib
            for ecp in (0, 2):
                ps = pjps.tile([P, 2, IB], F32, tag="pj")
                for g in (0, 1):
                    dr6(ps[:, g, :], m12_8, [ts(ecp + g, P)], xT8, [ts(ib, IB)])
                    hilo(qmT8, [ecp + g, ts(ib, IB)], ps[:, g, :])

        # DMA queue: V0a, V0b, W3, V1, W1, V2, W2, V3, X0
        nat_v = [loadp.tile([P, 4, D], F32R, tag="nat", name=f"nat_v{g}")
                 for g in range(2)]
        nc.sync.dma_start(nat_v[0][:, 0:1, :],
                          v_d[ts(0, P), :].rearrange("(c p) d -> p c d", p=P))
        nc.sync.dma_start(nat_v[0][:, 1:4, :],
                          v_d[P:4 * P, :].rearrange("(c p) d -> p c d", p=P))
        nc.sync.dma_start(w3n, w3_d.rearrange("(c p) u -> p c u", p=P))
        nc.sync.dma_start(nat_v[1],
                          v_d[ts(1, 4 * P), :].rearrange("(c p) d -> p c d", p=P))
        nc.sync.dma_start(w1n, w1_d.rearrange("(c p) u -> p c u", p=P))

        vn_done = 0
        for jcp in range(SC // 2):
            jc = 2 * jcp
            g = jc // 4
            if jc % 4 == 2 and g + 2 < 4:
                t = loadp.tile([P, 4, D], F32R, tag="nat", name=f"nat_v{g + 2}")
                nc.sync.dma_start(
                    t, v_d[ts(g + 2, 4 * P), :].rearrange("(c p) d -> p c d", p=P))
                nat_v.append(t)
                if g == 0:
                    nc.sync.dma_start(w2n, w2_d.rearrange("(c p) u -> p c u", p=P))
            transpose_pair(nat_v[g][:, jc % 4, :], nat_v[g][:, jc % 4 + 1, :],
                           vT8, jc)
            if jc == 2:
                hilo(w3_8, [slice(None), slice(None)], w3n, scale=WSC)
            elif jc == 6:
                emit_wt(w1n, w1t)
            elif jc == 10:
                emit_wt(w2n, w2t)
            elif jc >= 12:
                emit_m12((jc - 12) // 2)
            if jc >= 4:
                emit_vn2(vn_done)
                vn_done += 2

        # X group 0 + qmT(0); groups 1..3 are deferred into phase 2
        nat_x = loadp.tile([P, 4, D], F32R, tag="nat", name="nat_x0")
        nc.sync.dma_start(nat_x,
                          q_d[ts(0, 4 * P), :].rearrange("(c p) d -> p c d", p=P))
        for sc in range(4):
            emit_xt(sc, nat_x[:, sc, :])
            if vn_done < SC:
                emit_vn2(vn_done)
                vn_done += 2
        emit_qmt(0)


def _phase2(nc, tc, q_d, o_d, ident, m12_8, vT8, xT8, qmT8, vN, hilo, dr6):
    with tc.tile_pool(name="expp", bufs=2) as expp, \
         tc.tile_pool(name="loadp2", bufs=2) as loadp2, \
         tc.tile_pool(name="outp", bufs=4) as outp, \
         tc.tile_pool(name="scps", bufs=2, space="PSUM") as scps, \
         tc.tile_pool(name="tps2", bufs=1, space="PSUM") as tps2, \
         tc.tile_pool(name="pjps2", bufs=1, space="PSUM") as pjps2, \
         tc.tile_pool(name="caps", bufs=2, space="PSUM") as caps, \
         tc.tile_pool(name="cbps", bufs=2, space="PSUM") as cbps:

        def emit_xt2(sc, nat):
            tp = tps2.tile([P, DC, P], F32R, tag="tp2")
            for dc in range(DC):
                nc.tensor.transpose(tp[:, dc, :], nat[:, ts(dc, P)], ident)
            hilo(xT8, [slice(None), ts(sc, P)], tp)

        def emit_qmt2(ib):
            for ec in range(DC):
                ps = pjps2.tile([P, IB], F32, tag="pj2")
                dr6(ps, m12_8, [ts(ec, P)], xT8, [ts(ib, IB)])
                hilo(qmT8, [ec, ts(ib, IB)], ps)

        def emit_scores(ib):
            expB = expp.tile([P, SC, IB], F32R, name="expB")
            for jc in range(SC):
                ps = scps.tile([P, IB], F32, tag="sc")
                n = 0
                for pc in (0, 2):
                    for sh, mh in HL3:
                        n += 1
                        nc.tensor.matmul(
                            ps,
                            vT8[:, sh, pc:pc + 2, ts(jc, P)],
                            qmT8[:, mh, pc:pc + 2, ts(ib, IB)],
                            start=(n == 1), stop=(n == 6), perf_mode=DR)
                nc.scalar.activation(expB[:, jc, :], ps, EXP, scale=SCALE / WSC)
            return expB

        def emit_ctx(expB, ib):
            for icc in range(ICC):
                i_glob = ib * ICC + icc
                psA = caps.tile([P, CA], F32, tag="ca")
                psB = cbps.tile([P, CB], F32, tag="cb")
                for jc in range(SC):
                    st = expB[:, jc, ts(icc, P)]
                    nc.tensor.matmul(psA, st, vN[:, jc, 0:CA],
                                     start=(jc == 0), stop=(jc == SC - 1))
                    nc.tensor.matmul(psB, st, vN[:, jc, CA:CA + CB],
                                     start=(jc == 0), stop=(jc == SC - 1))
                # den (= 16*sum_j exp) sits at psB col U-CA (=254)
                rec = outp.tile([P, 1], F32, tag="rec")
                nc.vector.reciprocal(rec, psB[:, U - CA:U - CA + 1])
                co = outp.tile([P, U], F32, tag="co")
                nc.vector.tensor_scalar_mul(co[:, 0:CA], psA, rec)
                nc.scalar.mul(co[:, CA:U], psB[:, 0:U - CA], rec)
                nc.sync.dma_start(o_d[ts(i_glob, P), :], co)

        prev = None
        for ib in range(NIB):
            if ib < NIB - 1:
                nat_x = loadp2.tile([P, 4, D], F32R, tag="natx",
                                    name=f"nat_x{ib + 1}")
                nc.sync.dma_start(
                    nat_x,
                    q_d[ts(ib + 1, 4 * P), :].rearrange("(c p) d -> p c d", p=P))
            expB = emit_scores(ib)
            if ib < NIB - 1:
                for k in range(4):
                    emit_xt2(4 * (ib + 1) + k, nat_x[:, k, :])
                emit_qmt2(ib + 1)
            if prev is not None:
                emit_ctx(*prev)
            prev = (expB, ib)
        emit_ctx(*prev)


_PROGRAM = None


def _get_program():
    global _PROGRAM
    if _PROGRAM is None:
        nc = bacc.Bacc("TRN2", target_bir_lowering=False, debug=False,
                       num_devices=B)
        q_d = nc.dram_tensor("query", (S, D), F32R, kind="ExternalInput").ap()
        v_d = nc.dram_tensor("value", (S, D), F32R, kind="ExternalInput").ap()
        w1_d = nc.dram_tensor("W1", (D, U), F32R, kind="ExternalInput").ap()
        w2_d = nc.dram_tensor("W2", (D, U), F32R, kind="ExternalInput").ap()
        w3_d = nc.dram_tensor("W3", (D, U), F32R, kind="ExternalInput").ap()
        o_d = nc.dram_tensor("out", (S, U), F32, kind="ExternalOutput").ap()
        with tile.TileContext(nc) as tc:
            _emit(nc, tc, q_d, v_d, w1_d, w2_d, w3_d, o_d)
        nc.compile()
        _PROGRAM = nc
    return _PROGRAM


def kernel(**inputs) -> np.ndarray:
    query = np.ascontiguousarray(inputs["query"], dtype=np.float32)
    value = np.ascontiguousarray(inputs["value"], dtype=np.float32)
    W1 = np.ascontiguousarray(inputs["W1"], dtype=np.float32)
    W2 = np.ascontiguousarray(inputs["W2"], dtype=np.float32)
    W3 = np.ascontiguousarray(inputs["W3"], dtype=np.float32)
    assert query.shape == (B, S, D) and value.shape == (B, S, D)

    nc = _get_program()
    in_maps = [
        {"query": query[b], "value": value[b], "W1": W1, "W2": W2, "W3": W3}
        for b in range(B)
    ]
    res = run_bass_kernel_spmd(nc, in_maps, core_ids=list(range(B)))
    return np.stack([res.results[b]["out"] for b in range(B)], axis=0)


# revision 12
# speedup vs baseline: 1.2403x; 1.0114x over previous
"""Trainium2 Bass kernel for single-head attention (B=8, S=2048, D=U=512).

Sharding: data-parallel over batch - one batch element per NeuronCore (8 cores).

Math: score = X W1 (V W2)^T / sqrt(U) = X M V^T with M = W1 W2^T folded once
per core (saves one full projection). context = softmax(score) (V W3).

Dataflow per core:
  Phase 1 (streamed with the serial DMA queue: V0a,V0b,W3,V1,W1,V2,W2,V3,X0):
    - V chunks PE-transposed (f32r) and quantized to fp8e4 hi/lo (vT8).
    - W1,W2 PE-transposed; M = W1 W2^T on PE, scaled by 16, fp8 hi/lo (m12_8).
      W3 scaled by 16, fp8 hi/lo (w3_8).
    - vN[j,u] = 16*(V W3) via fp8 DoubleRow (3-term hi/lo cross products),
      f32r, with column 512 = 16.0 (fused softmax denominator).
    - X group 0 transposed/quantized (xT8); qmT(0) = 16*(M^T x^T) via
      DoubleRow, quantized hi/lo (qmT8).
  Phase 2 per i-block ib (pipelined):
    scores(ib): scoresT[j,i] = sum_e vT[e,j] qmT[e,i] via fp8 DoubleRow;
      exp on ACT -> expB (f32r). X group ib+1 transpose + qmT(ib+1) are
      interleaved here (PE) so their vector ops land in phase-2 ACT/DVE slack.
    ctx(ib-1): ctx[i,u] (+den col) = sum_j expB[j,i] vN[j,u], f32r matmuls in
      two psum groups (258+256 cols); out = ctx * recip(den) -> DMA.

fp8 DoubleRow (operands paired along a leading free dim of 2) runs at 0.5
cycles/output-col with 256-deep contraction per call - 4x f32r throughput;
the 3-call hi/lo scheme nets 2.67x at ~8-bit per-term accuracy. Measured
end-to-end rel err ~3e-3 (threshold 2e-2).
"""

import math
import os
import sys

for _p in ("/opt/trn_rl_repo", os.path.expanduser("~/.axon_site/_ro/trn_rl_repo")):
    if os.path.isdir(_p) and _p not in sys.path:
        sys.path.insert(0, _p)

import numpy as np

import concourse.bass as bass
import concourse.tile as tile
from concourse import bacc, mybir
from concourse.bass import ts
from concourse.bass_utils import run_bass_kernel_spmd
from concourse.masks import make_identity

F32 = mybir.dt.float32
F32R = mybir.dt.float32r
F8 = mybir.dt.float8e4
EXP = mybir.ActivationFunctionType.Exp
DR = mybir.MatmulPerfMode.DoubleRow
MUL = mybir.AluOpType.mult
SUB = mybir.AluOpType.subtract

P = 128          # partitions
B = 8            # batch (one element per core)
S = 2048         # sequence length
D = 512          # model dim
U = 512          # units
DC = D // P      # 4 chunks of the contraction dims
SC = S // P      # 16 s-chunks
IB = 512         # i-block (query positions per attention block)
NIB = S // IB    # 4
ICC = IB // P    # 4 i-chunks per block
SCALE = 1.0 / math.sqrt(float(U))
WSC = 16.0       # weight pre-scale so fp8 quantization stays in normal range
VNF = 520        # vN free width: 512 u-cols + col 512 = WSC (den) + pad
CA = 258         # ctx psum group A columns (u 0..257)
CB = 256         # ctx psum group B columns (u 258..511, den at 254, pad)

# DoubleRow hi/lo call list: (stationary half, moving half)
HL3 = ((0, 0), (0, 1), (1, 0))


def _emit(nc, tc, q_d, v_d, w1_d, w2_d, w3_d, o_d):
    with tc.tile_pool(name="const", bufs=1) as cp, \
         tc.tile_pool(name="persist", bufs=1) as pp:
        identf = cp.tile([P, P], F32, name="identf")
        make_identity(nc, identf)
        ident = cp.tile([P, P], F32R, name="ident")
        nc.vector.tensor_copy(ident, identf)

        m12_8 = pp.tile([P, 2, DC, D], F8, name="m12_8")
        w3_8 = pp.tile([P, 2, DC, U], F8, name="w3_8")
        vT8 = pp.tile([P, 2, DC, S], F8, name="vT8")
        xT8 = pp.tile([P, 2, DC, S], F8, name="xT8")
        qmT8 = pp.tile([P, 2, DC, S], F8, name="qmT8")
        vN = pp.tile([P, SC, VNF], F32R, name="vN")
        # den column = WSC (vN holds 16*v so num/den scales cancel); memset
        # on f32r fails ISA checks, so stage in f32 and copy.
        dtmp = cp.tile([P, SC, VNF - U], F32, name="dtmp")
        nc.gpsimd.memset(dtmp, 0.0)
        nc.gpsimd.memset(dtmp[:, :, 0:1], WSC)
        nc.vector.tensor_copy(vN[:, :, U:VNF], dtmp)

        _veng = [0]

        def hilo(dst8, hl_idx, src, scale=1.0):
            # dst8[...,0,...] = fp8(scale*src) on ACT;
            # dst8[...,1,...] = fp8(scale*src - hi) on DVE.
            hi = dst8[tuple([slice(None), 0] + hl_idx)]
            lo = dst8[tuple([slice(None), 1] + hl_idx)]
            if scale == 1.0:
                nc.scalar.copy(hi, src)
            else:
                nc.scalar.mul(hi, src, scale)
            nc.vector.scalar_tensor_tensor(lo, src, scale, hi, op0=MUL, op1=SUB)

        def copy_f32r(dst, src):
            _veng[0] += 1
            if _veng[0] % 2:
                nc.scalar.copy(dst, src)
            else:
                nc.vector.tensor_copy(dst, src)

        def dr6(ps, stat8, s_idx, mov8, m_idx):
            # 6 DoubleRow calls: 2 pair-chunks x (hh, hl, lh)
            n = 0
            for pc in (0, 2):
                for sh, mh in HL3:
                    n += 1
                    nc.tensor.matmul(
                        ps,
                        stat8[tuple([slice(None), sh, slice(pc, pc + 2)] + s_idx)],
                        mov8[tuple([slice(None), mh, slice(pc, pc + 2)] + m_idx)],
                        start=(n == 1), stop=(n == 6), perf_mode=DR)

        _phase1(nc, tc, q_d, v_d, w1_d, w2_d, w3_d, ident,
                m12_8, w3_8, vT8, xT8, qmT8, vN, hilo, copy_f32r, dr6)
        _phase2(nc, tc, q_d, o_d, ident, m12_8, vT8, xT8, qmT8, vN, hilo, dr6,
                copy_f32r, w3_8)


def _phase1(nc, tc, q_d, v_d, w1_d, w2_d, w3_d, ident,
            m12_8, w3_8, vT8, xT8, qmT8, vN, hilo, copy_f32r, dr6):
    with tc.tile_pool(name="wtmp", bufs=1) as wp, \
         tc.tile_pool(name="loadp", bufs=3) as loadp, \
         tc.tile_pool(name="tps", bufs=2, space="PSUM") as tpsp, \
         tc.tile_pool(name="pjps", bufs=2, space="PSUM") as pjps:
        w1n = wp.tile([P, DC, U], F32R, name="w1n")
        w2n = wp.tile([P, DC, U], F32R, name="w2n")
        w3n = wp.tile([P, DC, U], F32R, name="w3n")
        w1t = wp.tile([P, DC, D], F32R, name="w1t")
        w2t = wp.tile([P, DC, D], F32R, name="w2t")

        def transpose_pair(nat0, nat1, dst8, jc):
            # two 128-row chunks -> one 2-bank psum tile (layout [c, g, q] so
            # (g q) merges to a contiguous 256-wide dim) -> one hi + one lo op
            tp = tpsp.tile([P, DC, 2, P], F32R, tag="tp")
            for g, nat in ((0, nat0), (1, nat1)):
                for dc in range(DC):
                    nc.tensor.transpose(tp[:, dc, g, :], nat[:, ts(dc, P)], ident)
            src = tp.rearrange("p c g q -> p c (g q)")
            hi = dst8[:, 0, :, jc * P:(jc + 2) * P]
            lo = dst8[:, 1, :, jc * P:(jc + 2) * P]
            nc.scalar.copy(hi, src)
            nc.vector.scalar_tensor_tensor(lo, src, 1.0, hi, op0=MUL, op1=SUB)

        def emit_wt(wn, wt):
            # wt[u%128, uc, d] = W[d, u] transposed blocks
            for ucp in (0, 2):
                tp = tpsp.tile([P, DC, 2, P], F32R, tag="tp")
                for g in (0, 1):
                    for dc in range(DC):
                        nc.tensor.transpose(tp[:, dc, g, :],
                                            wn[:, dc, ts(ucp + g, P)], ident)
                    copy_f32r(
                        wt[:, ucp + g, :].rearrange("p (k q) -> p k q", k=DC),
                        tp[:, :, g, :])

        def emit_m12(dcp):
            # M[d,e] = sum_u W1[d,u] W2[e,u], row-chunks (2dcp, 2dcp+1)
            ps = pjps.tile([P, 2, D], F32, tag="pj")
            for g in (0, 1):
                for uc in range(DC):
                    nc.tensor.matmul(ps[:, g, :], w1t[:, uc, ts(2 * dcp + g, P)],
                                     w2t[:, uc, :],
                                     start=(uc == 0), stop=(uc == DC - 1))
                hilo(m12_8, [2 * dcp + g, slice(None)], ps[:, g, :], scale=WSC)

        def emit_vn2(jc):
            # vN[j,u] = 16 * sum_d V[j,d] W3[d,u], two j-chunks at once
            ps = pjps.tile([P, 2, U], F32, tag="pj")
            dr6(ps[:, 0, :], vT8, [ts(jc, P)], w3_8, [slice(None)])
            dr6(ps[:, 1, :], vT8, [ts(jc + 1, P)], w3_8, [slice(None)])
            copy_f32r(vN[:, jc:jc + 2, 0:U], ps)

        def emit_xt(sc, nat):
            tp = tpsp.tile([P, DC, 2, P], F32R, tag="tp")
            for dc in range(DC):
                nc.tensor.transpose(tp[:, dc, 0, :], nat[:, ts(dc, P)], ident)
            hilo(xT8, [slice(None), ts(sc, P)], tp[:, :, 0, :])

        def emit_qmt(ib):
            # qmT[e,i] = 16 * sum_d M[d,e] xT[d,i] for i-block ib
            for ecp in (0, 2):
                ps = pjps.tile([P, 2, IB], F32, tag="pj")
                for g in (0, 1):
                    dr6(ps[:, g, :], m12_8, [ts(ecp + g, P)], xT8, [ts(ib, IB)])
                    hilo(qmT8, [ecp + g, ts(ib, IB)], ps[:, g, :])

        # DMA queue: V0a(chunk0), W3, V0b(1-3), V1, W1, V2, W2, V3, X0.
        # W3 right after the first chunk so vN matmuls can start early; the
        # vN stream then fills every later DMA-arrival stall.
        nat_v = [loadp.tile([P, 4, D], F32R, tag="nat", name=f"nat_v{g}")
                 for g in range(2)]
        nc.sync.dma_start(nat_v[0][:, 0:1, :],
                          v_d[ts(0, P), :].rearrange("(c p) d -> p c d", p=P))
        nc.sync.dma_start(w3n, w3_d.rearrange("(c p) u -> p c u", p=P))
        nc.sync.dma_start(nat_v[0][:, 1:4, :],
                          v_d[P:4 * P, :].rearrange("(c p) d -> p c d", p=P))
        nc.sync.dma_start(nat_v[1],
                          v_d[ts(1, 4 * P), :].rearrange("(c p) d -> p c d", p=P))
        nc.sync.dma_start(w1n, w1_d.rearrange("(c p) u -> p c u", p=P))

        def tp_single(jc, nat):
            tp = tpsp.tile([P, DC, 2, P], F32R, tag="tp")
            for dc in range(DC):
                nc.tensor.transpose(tp[:, dc, 0, :], nat[:, ts(dc, P)], ident)
            hilo(vT8, [slice(None), ts(jc, P)], tp[:, :, 0, :])

        tp_single(0, nat_v[0][:, 0, :])
        hilo(w3_8, [slice(None), slice(None)], w3n, scale=WSC)
        transpose_pair(nat_v[0][:, 1, :], nat_v[0][:, 2, :], vT8, 1)
        tp_single(3, nat_v[0][:, 3, :])
        nat_v.append(None)
        nat_v.append(None)
        sched = [
            ("vn", 0), ("vn", 2),
            ("dma_v", 2), ("dma_w2", None),
            ("tp", 4), ("vn", 4), ("tp", 6),
            ("wt", 1), ("vn", 6),
            ("dma_v", 3),
            ("tp", 8), ("vn", 8), ("tp", 10),
            ("wt", 2), ("vn", 10),
            ("dma_x", 0),
            ("m12", 0), ("tp", 12), ("m12", 1), ("tp", 14),
            ("xt", 0), ("xt", 1), ("xt", 2), ("xt", 3),
            ("qmt", 0),
        ]
        nat_x = None
        for op, arg in sched:
            if op == "dma_v":
                t = loadp.tile([P, 4, D], F32R, tag="nat", name=f"nat_v{arg}")
                nc.sync.dma_start(
                    t, v_d[ts(arg, 4 * P), :].rearrange("(c p) d -> p c d", p=P))
                nat_v[arg] = t
            elif op == "dma_w2":
                nc.sync.dma_start(w2n, w2_d.rearrange("(c p) u -> p c u", p=P))
            elif op == "dma_x":
                nat_x = loadp.tile([P, 4, D], F32R, tag="nat", name="nat_x0")
                nc.sync.dma_start(
                    nat_x, q_d[ts(0, 4 * P), :].rearrange("(c p) d -> p c d", p=P))
            elif op == "tp":
                jc = arg
                g = jc // 4
                transpose_pair(nat_v[g][:, jc % 4, :], nat_v[g][:, jc % 4 + 1, :],
                               vT8, jc)
            elif op == "vn":
                emit_vn2(arg)
            elif op == "wt":
                emit_wt(w1n if arg == 1 else w2n, w1t if arg == 1 else w2t)
            elif op == "m12":
                emit_m12(arg)
            elif op == "xt":
                emit_xt(arg, nat_x[:, arg, :])
            elif op == "qmt":
                emit_qmt(0)
        # vN chunks 12..15 are deferred into phase 2


def _phase2(nc, tc, q_d, o_d, ident, m12_8, vT8, xT8, qmT8, vN, hilo, dr6,
            copy_f32r, w3_8):
    # One shared scratch-psum pool (4 x 512-f32 banks) serves score groups,
    # deferred X transposes, qmT and vN tail work; ctx keeps 2+2 banks.
    with tc.tile_pool(name="expp", bufs=2) as expp, \
         tc.tile_pool(name="loadp2", bufs=2) as loadp2, \
         tc.tile_pool(name="outp", bufs=4) as outp, \
         tc.tile_pool(name="wkps", bufs=4, space="PSUM") as wkps, \
         tc.tile_pool(name="caps", bufs=2, space="PSUM") as caps, \
         tc.tile_pool(name="cbps", bufs=2, space="PSUM") as cbps:

        def emit_xt2(sc, nat):
            ps = wkps.tile([P, IB], F32, tag="wk")
            tpv = ps.bitcast(F32R).rearrange("p (c q) -> p c q", c=DC)
            for dc in range(DC):
                nc.tensor.transpose(tpv[:, dc, :], nat[:, ts(dc, P)], ident)
            hilo(xT8, [slice(None), ts(sc, P)], tpv)

        def emit_qmt2(ib):
            for ec in range(DC):
                ps = wkps.tile([P, IB], F32, tag="wk")
                dr6(ps, m12_8, [ts(ec, P)], xT8, [ts(ib, IB)])
                hilo(qmT8, [ec, ts(ib, IB)], ps)

        def emit_vn1(jc):
            ps = wkps.tile([P, IB], F32, tag="wk")
            dr6(ps, vT8, [ts(jc, P)], w3_8, [slice(None)])
            copy_f32r(vN[:, jc, 0:U], ps)

        def emit_scores(ib):
            expB = expp.tile([P, SC, IB], F32R, name="expB")
            for jc in range(SC):
                ps = wkps.tile([P, IB], F32, tag="wk")
                n = 0
                for pc in (0, 2):
                    for sh, mh in HL3:
                        n += 1
                        nc.tensor.matmul(
                            ps,
                            vT8[:, sh, pc:pc + 2, ts(jc, P)],
                            qmT8[:, mh, pc:pc + 2, ts(ib, IB)],
                            start=(n == 1), stop=(n == 6), perf_mode=DR)
                nc.scalar.activation(expB[:, jc, :], ps, EXP, scale=SCALE / WSC)
                if ib == 0 and jc in (5, 7, 9, 11):
                    emit_vn1(12 + (jc - 5) // 2)
            return expB

        def emit_ctx(expB, ib):
            for icc in range(ICC):
                i_glob = ib * ICC + icc
                psA = caps.tile([P, CA], F32, tag="ca")
                psB = cbps.tile([P, CB], F32, tag="cb")
                for jc in range(SC):
                    st = expB[:, jc, ts(icc, P)]
                    nc.tensor.matmul(psA, st, vN[:, jc, 0:CA],
                                     start=(jc == 0), stop=(jc == SC - 1))
                    nc.tensor.matmul(psB, st, vN[:, jc, CA:CA + CB],
                                     start=(jc == 0), stop=(jc == SC - 1))
                # den (= 16*sum_j exp) sits at psB col U-CA (=254)
                rec = outp.tile([P, 1], F32, tag="rec")
                nc.vector.reciprocal(rec, psB[:, U - CA:U - CA + 1])
                co = outp.tile([P, U], F32, tag="co")
                nc.vector.tensor_scalar_mul(co[:, 0:CA], psA, rec)
                nc.sync.dma_start(o_d[ts(i_glob, P), 0:CA], co[:, 0:CA])
                nc.scalar.mul(co[:, CA:U], psB[:, 0:U - CA], rec)
                nc.sync.dma_start(o_d[ts(i_glob, P), CA:U], co[:, CA:U])

        prev = None
        for ib in range(NIB):
            if ib < NIB - 1:
                nat_x = loadp2.tile([P, 4, D], F32R, tag="natx",
                                    name=f"nat_x{ib + 1}")
                nc.sync.dma_start(
                    nat_x,
                    q_d[ts(ib + 1, 4 * P), :].rearrange("(c p) d -> p c d", p=P))
            expB = emit_scores(ib)
            if ib < NIB - 1:
                for k in range(4):
                    emit_xt2(4 * (ib + 1) + k, nat_x[:, k, :])
                emit_qmt2(ib + 1)
            if prev is not None:
                emit_ctx(*prev)
            prev = (expB, ib)
        emit_ctx(*prev)


_PROGRAM = None


def _get_program():
    global _PROGRAM
    if _PROGRAM is None:
        nc = bacc.Bacc("TRN2", target_bir_lowering=False, debug=False,
                       num_devices=B)
        q_d = nc.dram_tensor("query", (S, D), F32R, kind="ExternalInput").ap()
        v_d = nc.dram_tensor("value", (S, D), F32R, kind="ExternalInput").ap()
        w1_d = nc.dram_tensor("W1", (D, U), F32R, kind="ExternalInput").ap()
        w2_d = nc.dram_tensor("W2", (D, U), F32R, kind="ExternalInput").ap()
        w3_d = nc.dram_tensor("W3", (D, U), F32R, kind="ExternalInput").ap()
        o_d = nc.dram_tensor("out", (S, U), F32, kind="ExternalOutput").ap()
        with tile.TileContext(nc) as tc:
            _emit(nc, tc, q_d, v_d, w1_d, w2_d, w3_d, o_d)
        nc.compile()
        _PROGRAM = nc
    return _PROGRAM


def kernel(**inputs) -> np.ndarray:
    query = np.ascontiguousarray(inputs["query"], dtype=np.float32)
    value = np.ascontiguousarray(inputs["value"], dtype=np.float32)
    W1 = np.ascontiguousarray(inputs["W1"], dtype=np.float32)
    W2 = np.ascontiguousarray(inputs["W2"], dtype=np.float32)
    W3 = np.ascontiguousarray(inputs["W3"], dtype=np.float32)
    assert query.shape == (B, S, D) and value.shape == (B, S, D)

    nc = _get_program()
    in_maps = [
        {"query": query[b], "value": value[b], "W1": W1, "W2": W2, "W3": W3}
        for b in range(B)
    ]
    res = run_bass_kernel_spmd(nc, in_maps, core_ids=list(range(B)))
    return np.stack([res.results[b]["out"] for b in range(B)], axis=0)


# revision 13
# speedup vs baseline: 1.2504x; 1.0082x over previous
"""Trainium2 Bass kernel for single-head attention (B=8, S=2048, D=U=512).

Sharding: data-parallel over batch - one batch element per NeuronCore (8 cores).

Math: score = X W1 (V W2)^T / sqrt(U) = X M V^T with M = W1 W2^T folded once
per core (saves one full projection). context = softmax(score) (V W3).

Dataflow per core:
  Phase 1 (streamed with the serial DMA queue: V0a,V0b,W3,V1,W1,V2,W2,V3,X0):
    - V chunks PE-transposed (f32r) and quantized to fp8e4 hi/lo (vT8).
    - W1,W2 PE-transposed; M = W1 W2^T on PE, scaled by 16, fp8 hi/lo (m12_8).
      W3 scaled by 16, fp8 hi/lo (w3_8).
    - vN[j,u] = 16*(V W3) via fp8 DoubleRow (3-term hi/lo cross products),
      f32r, with column 512 = 16.0 (fused softmax denominator).
    - X group 0 transposed/quantized (xT8); qmT(0) = 16*(M^T x^T) via
      DoubleRow, quantized hi/lo (qmT8).
  Phase 2 per i-block ib (pipelined):
    scores(ib): scoresT[j,i] = sum_e vT[e,j] qmT[e,i] via fp8 DoubleRow;
      exp on ACT -> expB (f32r). X group ib+1 transpose + qmT(ib+1) are
      interleaved here (PE) so their vector ops land in phase-2 ACT/DVE slack.
    ctx(ib-1): ctx[i,u] (+den col) = sum_j expB[j,i] vN[j,u], f32r matmuls in
      two psum groups (258+256 cols); out = ctx * recip(den) -> DMA.

fp8 DoubleRow (operands paired along a leading free dim of 2) runs at 0.5
cycles/output-col with 256-deep contraction per call - 4x f32r throughput;
the 3-call hi/lo scheme nets 2.67x at ~8-bit per-term accuracy. Measured
end-to-end rel err ~3e-3 (threshold 2e-2).
"""

import math
import os
import sys

for _p in ("/opt/trn_rl_repo", os.path.expanduser("~/.axon_site/_ro/trn_rl_repo")):
    if os.path.isdir(_p) and _p not in sys.path:
        sys.path.insert(0, _p)

import numpy as np

import concourse.bass as bass
import concourse.tile as tile
from concourse import bacc, mybir
from concourse.bass import ts
from concourse.bass_utils import run_bass_kernel_spmd
from concourse.masks import make_identity

F32 = mybir.dt.float32
F32R = mybir.dt.float32r
F8 = mybir.dt.float8e4
EXP = mybir.ActivationFunctionType.Exp
DR = mybir.MatmulPerfMode.DoubleRow
MUL = mybir.AluOpType.mult
SUB = mybir.AluOpType.subtract

P = 128          # partitions
B = 8            # batch (one element per core)
S = 2048         # sequence length
D = 512          # model dim
U = 512          # units
DC = D // P      # 4 chunks of the contraction dims
SC = S // P      # 16 s-chunks
IB = 512         # i-block (query positions per attention block)
NIB = S // IB    # 4
ICC = IB // P    # 4 i-chunks per block
SCALE = 1.0 / math.sqrt(float(U))
WSC = 16.0       # weight pre-scale so fp8 quantization stays in normal range
VNF = 520        # vN free width: 512 u-cols + col 512 = WSC (den) + pad
CA = 258         # ctx psum group A columns (u 0..257)
CB = 256         # ctx psum group B columns (u 258..511, den at 254, pad)

# DoubleRow hi/lo call list: (stationary half, moving half)
HL3 = ((0, 0), (0, 1), (1, 0))


def _emit(nc, tc, q_d, v_d, w1_d, w2_d, w3_d, o_d):
    with tc.tile_pool(name="const", bufs=1) as cp, \
         tc.tile_pool(name="persist", bufs=1) as pp:
        identf = cp.tile([P, P], F32, name="identf")
        make_identity(nc, identf)
        ident = cp.tile([P, P], F32R, name="ident")
        nc.vector.tensor_copy(ident, identf)

        m12_8 = pp.tile([P, 2, DC, D], F8, name="m12_8")
        w3_8 = pp.tile([P, 2, DC, U], F8, name="w3_8")
        vT8 = pp.tile([P, 2, DC, S], F8, name="vT8")
        xT8 = pp.tile([P, 2, DC, S], F8, name="xT8")
        qmT8 = pp.tile([P, 2, DC, S], F8, name="qmT8")
        vN = pp.tile([P, SC, VNF], F32R, name="vN")
        # den column = WSC (vN holds 16*v so num/den scales cancel); memset
        # on f32r fails ISA checks, so stage in f32 and copy.
        dtmp = cp.tile([P, SC, VNF - U], F32, name="dtmp")
        nc.gpsimd.memset(dtmp, 0.0)
        nc.gpsimd.memset(dtmp[:, :, 0:1], WSC)
        nc.vector.tensor_copy(vN[:, :, U:VNF], dtmp)

        _veng = [0]

        def hilo(dst8, hl_idx, src, scale=1.0, dve_hi=False):
            # dst8[...,0,...] = fp8(scale*src) on ACT (or DVE if dve_hi);
            # dst8[...,1,...] = fp8(scale*src - hi) on DVE.
            hi = dst8[tuple([slice(None), 0] + hl_idx)]
            lo = dst8[tuple([slice(None), 1] + hl_idx)]
            if dve_hi:
                nc.vector.tensor_copy(hi, src)
            elif scale == 1.0:
                nc.scalar.copy(hi, src)
            else:
                nc.scalar.mul(hi, src, scale)
            nc.vector.scalar_tensor_tensor(lo, src, scale, hi, op0=MUL, op1=SUB)

        def copy_f32r(dst, src):
            _veng[0] += 1
            if _veng[0] % 2:
                nc.scalar.copy(dst, src)
            else:
                nc.vector.tensor_copy(dst, src)

        def dr6(ps, stat8, s_idx, mov8, m_idx):
            # 6 DoubleRow calls: 2 pair-chunks x (hh, hl, lh)
            n = 0
            for pc in (0, 2):
                for sh, mh in HL3:
                    n += 1
                    nc.tensor.matmul(
                        ps,
                        stat8[tuple([slice(None), sh, slice(pc, pc + 2)] + s_idx)],
                        mov8[tuple([slice(None), mh, slice(pc, pc + 2)] + m_idx)],
                        start=(n == 1), stop=(n == 6), perf_mode=DR)

        _phase1(nc, tc, q_d, v_d, w1_d, w2_d, w3_d, ident,
                m12_8, w3_8, vT8, xT8, qmT8, vN, hilo, copy_f32r, dr6)
        _phase2(nc, tc, q_d, o_d, ident, m12_8, vT8, xT8, qmT8, vN, hilo, dr6,
                copy_f32r, w3_8)


def _phase1(nc, tc, q_d, v_d, w1_d, w2_d, w3_d, ident,
            m12_8, w3_8, vT8, xT8, qmT8, vN, hilo, copy_f32r, dr6):
    with tc.tile_pool(name="wtmp", bufs=1) as wp, \
         tc.tile_pool(name="loadp", bufs=3) as loadp, \
         tc.tile_pool(name="tps", bufs=2, space="PSUM") as tpsp, \
         tc.tile_pool(name="pjps", bufs=2, space="PSUM") as pjps:
        w1n = wp.tile([P, DC, U], F32R, name="w1n")
        w2n = wp.tile([P, DC, U], F32R, name="w2n")
        w3n = wp.tile([P, DC, U], F32R, name="w3n")
        w1t = wp.tile([P, DC, D], F32R, name="w1t")
        w2t = wp.tile([P, DC, D], F32R, name="w2t")

        def transpose_pair(nat0, nat1, dst8, jc):
            # two 128-row chunks -> one 2-bank psum tile (layout [c, g, q] so
            # (g q) merges to a contiguous 256-wide dim) -> one hi + one lo op
            tp = tpsp.tile([P, DC, 2, P], F32R, tag="tp")
            for g, nat in ((0, nat0), (1, nat1)):
                for dc in range(DC):
                    nc.tensor.transpose(tp[:, dc, g, :], nat[:, ts(dc, P)], ident)
            src = tp.rearrange("p c g q -> p c (g q)")
            hi = dst8[:, 0, :, jc * P:(jc + 2) * P]
            lo = dst8[:, 1, :, jc * P:(jc + 2) * P]
            nc.scalar.copy(hi, src)
            nc.vector.scalar_tensor_tensor(lo, src, 1.0, hi, op0=MUL, op1=SUB)

        def emit_wt(wn, wt):
            # wt[u%128, uc, d] = W[d, u] transposed blocks
            for ucp in (0, 2):
                tp = tpsp.tile([P, DC, 2, P], F32R, tag="tp")
                for g in (0, 1):
                    for dc in range(DC):
                        nc.tensor.transpose(tp[:, dc, g, :],
                                            wn[:, dc, ts(ucp + g, P)], ident)
                    copy_f32r(
                        wt[:, ucp + g, :].rearrange("p (k q) -> p k q", k=DC),
                        tp[:, :, g, :])

        def emit_m12(dcp):
            # M[d,e] = sum_u W1[d,u] W2[e,u], row-chunks (2dcp, 2dcp+1)
            ps = pjps.tile([P, 2, D], F32, tag="pj")
            for g in (0, 1):
                for uc in range(DC):
                    nc.tensor.matmul(ps[:, g, :], w1t[:, uc, ts(2 * dcp + g, P)],
                                     w2t[:, uc, :],
                                     start=(uc == 0), stop=(uc == DC - 1))
                hilo(m12_8, [2 * dcp + g, slice(None)], ps[:, g, :], scale=WSC)

        def emit_vn2(jc):
            # vN[j,u] = 16 * sum_d V[j,d] W3[d,u], two j-chunks at once
            ps = pjps.tile([P, 2, U], F32, tag="pj")
            dr6(ps[:, 0, :], vT8, [ts(jc, P)], w3_8, [slice(None)])
            dr6(ps[:, 1, :], vT8, [ts(jc + 1, P)], w3_8, [slice(None)])
            copy_f32r(vN[:, jc:jc + 2, 0:U], ps)

        def emit_xt(sc, nat):
            tp = tpsp.tile([P, DC, 2, P], F32R, tag="tp")
            for dc in range(DC):
                nc.tensor.transpose(tp[:, dc, 0, :], nat[:, ts(dc, P)], ident)
            hilo(xT8, [slice(None), ts(sc, P)], tp[:, :, 0, :])

        def emit_qmt(ib):
            # qmT[e,i] = 16 * sum_d M[d,e] xT[d,i] for i-block ib
            for ecp in (0, 2):
                ps = pjps.tile([P, 2, IB], F32, tag="pj")
                for g in (0, 1):
                    dr6(ps[:, g, :], m12_8, [ts(ecp + g, P)], xT8, [ts(ib, IB)])
                    hilo(qmT8, [ecp + g, ts(ib, IB)], ps[:, g, :])

        # DMA queue: V0a(chunk0), W3, V0b(1-3), V1, W1, V2, W2, V3, X0.
        # W3 right after the first chunk so vN matmuls can start early; the
        # vN stream then fills every later DMA-arrival stall.
        nat_v = [loadp.tile([P, 4, D], F32R, tag="nat", name=f"nat_v{g}")
                 for g in range(2)]
        for c in range(4):
            nc.sync.dma_start(
                nat_v[0][:, c:c + 1, :],
                v_d[ts(c, P), :].rearrange("(c p) d -> p c d", p=P))
        nc.sync.dma_start(w3n, w3_d.rearrange("(c p) u -> p c u", p=P))
        nc.sync.dma_start(nat_v[1],
                          v_d[ts(1, 4 * P), :].rearrange("(c p) d -> p c d", p=P))
        nc.sync.dma_start(w1n, w1_d.rearrange("(c p) u -> p c u", p=P))

        def tp_single(jc, nat):
            tp = tpsp.tile([P, DC, 2, P], F32R, tag="tp")
            for dc in range(DC):
                nc.tensor.transpose(tp[:, dc, 0, :], nat[:, ts(dc, P)], ident)
            hilo(vT8, [slice(None), ts(jc, P)], tp[:, :, 0, :])

        for c in range(4):
            tp_single(c, nat_v[0][:, c, :])
        hilo(w3_8, [slice(None), slice(None)], w3n, scale=WSC)
        nat_v.append(None)
        nat_v.append(None)
        sched = [
            ("vn", 0), ("vn", 2),
            ("dma_v", 2), ("dma_w2", None),
            ("tp", 4), ("vn", 4), ("tp", 6),
            ("wt", 1), ("vn", 6),
            ("dma_v", 3),
            ("tp", 8), ("vn", 8), ("tp", 10),
            ("wt", 2), ("vn", 10),
            ("dma_x", 0),
            ("m12", 0), ("tp", 12), ("m12", 1), ("tp", 14),
            ("xt", 0), ("xt", 1), ("xt", 2), ("xt", 3),
            ("qmt", 0),
        ]
        nat_x = None
        for op, arg in sched:
            if op == "dma_v":
                t = loadp.tile([P, 4, D], F32R, tag="nat", name=f"nat_v{arg}")
                nc.sync.dma_start(
                    t, v_d[ts(arg, 4 * P), :].rearrange("(c p) d -> p c d", p=P))
                nat_v[arg] = t
            elif op == "dma_w2":
                nc.sync.dma_start(w2n, w2_d.rearrange("(c p) u -> p c u", p=P))
            elif op == "dma_x":
                nat_x = loadp.tile([P, 4, D], F32R, tag="nat", name="nat_x0")
                nc.sync.dma_start(
                    nat_x, q_d[ts(0, 4 * P), :].rearrange("(c p) d -> p c d", p=P))
            elif op == "tp":
                jc = arg
                g = jc // 4
                transpose_pair(nat_v[g][:, jc % 4, :], nat_v[g][:, jc % 4 + 1, :],
                               vT8, jc)
            elif op == "vn":
                emit_vn2(arg)
            elif op == "wt":
                emit_wt(w1n if arg == 1 else w2n, w1t if arg == 1 else w2t)
            elif op == "m12":
                emit_m12(arg)
            elif op == "xt":
                emit_xt(arg, nat_x[:, arg, :])
            elif op == "qmt":
                emit_qmt(0)
        # vN chunks 12..15 are deferred into phase 2


def _phase2(nc, tc, q_d, o_d, ident, m12_8, vT8, xT8, qmT8, vN, hilo, dr6,
            copy_f32r, w3_8):
    # One shared scratch-psum pool (4 x 512-f32 banks) serves score groups,
    # deferred X transposes, qmT and vN tail work; ctx keeps 2+2 banks.
    with tc.tile_pool(name="expp", bufs=2) as expp, \
         tc.tile_pool(name="loadp2", bufs=2) as loadp2, \
         tc.tile_pool(name="outp", bufs=4) as outp, \
         tc.tile_pool(name="wkps", bufs=4, space="PSUM") as wkps, \
         tc.tile_pool(name="caps", bufs=2, space="PSUM") as caps, \
         tc.tile_pool(name="cbps", bufs=2, space="PSUM") as cbps:

        def emit_xt2(sc, nat):
            ps = wkps.tile([P, IB], F32, tag="wk")
            tpv = ps.bitcast(F32R).rearrange("p (c q) -> p c q", c=DC)
            for dc in range(DC):
                nc.tensor.transpose(tpv[:, dc, :], nat[:, ts(dc, P)], ident)
            hilo(xT8, [slice(None), ts(sc, P)], tpv, dve_hi=True)

        def emit_qmt2(ib):
            for ec in range(DC):
                ps = wkps.tile([P, IB], F32, tag="wk")
                dr6(ps, m12_8, [ts(ec, P)], xT8, [ts(ib, IB)])
                hilo(qmT8, [ec, ts(ib, IB)], ps, dve_hi=True)

        def emit_vn1(jc):
            ps = wkps.tile([P, IB], F32, tag="wk")
            dr6(ps, vT8, [ts(jc, P)], w3_8, [slice(None)])
            nc.vector.tensor_copy(vN[:, jc, 0:U], ps)

        def emit_scores(ib):
            expB = expp.tile([P, SC, IB], F32R, name="expB")
            for jc in range(SC):
                ps = wkps.tile([P, IB], F32, tag="wk")
                n = 0
                for pc in (0, 2):
                    for sh, mh in HL3:
                        n += 1
                        nc.tensor.matmul(
                            ps,
                            vT8[:, sh, pc:pc + 2, ts(jc, P)],
                            qmT8[:, mh, pc:pc + 2, ts(ib, IB)],
                            start=(n == 1), stop=(n == 6), perf_mode=DR)
                nc.scalar.activation(expB[:, jc, :], ps, EXP, scale=SCALE / WSC)
                if ib == 0 and jc in (5, 7, 9, 11):
                    emit_vn1(12 + (jc - 5) // 2)
            return expB

        def emit_ctx(expB, ib):
            for icc in range(ICC):
                i_glob = ib * ICC + icc
                psA = caps.tile([P, CA], F32, tag="ca")
                psB = cbps.tile([P, CB], F32, tag="cb")
                for jc in range(SC):
                    st = expB[:, jc, ts(icc, P)]
                    nc.tensor.matmul(psA, st, vN[:, jc, 0:CA],
                                     start=(jc == 0), stop=(jc == SC - 1))
                    nc.tensor.matmul(psB, st, vN[:, jc, CA:CA + CB],
                                     start=(jc == 0), stop=(jc == SC - 1))
                # den (= 16*sum_j exp) sits at psB col U-CA (=254)
                rec = outp.tile([P, 1], F32, tag="rec")
                nc.vector.reciprocal(rec, psB[:, U - CA:U - CA + 1])
                co = outp.tile([P, U], F32, tag="co")
                nc.vector.tensor_scalar_mul(co[:, 0:CA], psA, rec)
                nc.sync.dma_start(o_d[ts(i_glob, P), 0:CA], co[:, 0:CA])
                nc.scalar.mul(co[:, CA:U], psB[:, 0:U - CA], rec)
                nc.sync.dma_start(o_d[ts(i_glob, P), CA:U], co[:, CA:U])

        prev = None
        for ib in range(NIB):
            if ib < NIB - 1:
                nat_x = loadp2.tile([P, 4, D], F32R, tag="natx",
                                    name=f"nat_x{ib + 1}")
                nc.sync.dma_start(
                    nat_x,
                    q_d[ts(ib + 1, 4 * P), :].rearrange("(c p) d -> p c d", p=P))
            expB = emit_scores(ib)
            if ib < NIB - 1:
                for k in range(4):
                    emit_xt2(4 * (ib + 1) + k, nat_x[:, k, :])
                emit_qmt2(ib + 1)
            if prev is not None:
                emit_ctx(*prev)
            prev = (expB, ib)
        emit_ctx(*prev)


_PROGRAM = None


def _get_program():
    global _PROGRAM
    if _PROGRAM is None:
        nc = bacc.Bacc("TRN2", target_bir_lowering=False, debug=False,
                       num_devices=B)
        q_d = nc.dram_tensor("query", (S, D), F32R, kind="ExternalInput").ap()
        v_d = nc.dram_tensor("value", (S, D), F32R, kind="ExternalInput").ap()
        w1_d = nc.dram_tensor("W1", (D, U), F32R, kind="ExternalInput").ap()
        w2_d = nc.dram_tensor("W2", (D, U), F32R, kind="ExternalInput").ap()
        w3_d = nc.dram_tensor("W3", (D, U), F32R, kind="ExternalInput").ap()
        o_d = nc.dram_tensor("out", (S, U), F32, kind="ExternalOutput").ap()
        with tile.TileContext(nc) as tc:
            _emit(nc, tc, q_d, v_d, w1_d, w2_d, w3_d, o_d)
        nc.compile()
        _PROGRAM = nc
    return _PROGRAM


def kernel(**inputs) -> np.ndarray:
    query = np.ascontiguousarray(inputs["query"], dtype=np.float32)
    value = np.ascontiguousarray(inputs["value"], dtype=np.float32)
    W1 = np.ascontiguousarray(inputs["W1"], dtype=np.float32)
    W2 = np.ascontiguousarray(inputs["W2"], dtype=np.float32)
    W3 = np.ascontiguousarray(inputs["W3"], dtype=np.float32)
    assert query.shape == (B, S, D) and value.shape == (B, S, D)

    nc = _get_program()
    in_maps = [
        {"query": query[b], "value": value[b], "W1": W1, "W2": W2, "W3": W3}
        for b in range(B)
    ]
    res = run_bass_kernel_spmd(nc, in_maps, core_ids=list(range(B)))
    return np.stack([res.results[b]["out"] for b in range(B)], axis=0)
